# revision 60
# baseline (speedup 1.0000x reference)
"""Multi-head attention + out-proj + residual + LayerNorm on 8 trn2 cores.

Sharding: (batch, seq-half) -> 8 shards, collective-free. Each core gets
transposed activations (host-prepped) plus shared (transposed) weights and
computes its full [1024, 1024] output block.

Design (764us -> 413us -> this): the kernel is ACT(exp)-paced: 256 exp
ACTIVATEs of N=1024 at ~1.01us back-to-back floor = ~260us stream. Every
thing else must hide under it.  v2 changes vs the 413us baseline:
  * all projections (K/Q/V) + the final out-proj run fp8e4 DoubleRow
    (256-deep contraction per MM, 2 elem/cycle moving stream) -> PE work
    drops ~354us -> ~230us so the pumped proj groups no longer stretch
    the exp stream.
  * host ships activations/weights as fp8 (halves the DMA critical path).
    Residual + LN path stays bf16; kt/qt score operands stay bf16.
  * boundary hoisting: each attn call emits the NEXT attn's first scores
    matmul right after its last exp and before its last PV pair, so the
    PE FIFO never head-of-line blocks the next exp at call boundaries.

  phase V: V_all[Sk, H, 1+dv] fp8 in SBUF (ones col first = softmax denom)
  loop c (head pair): K/Q proj for c+1 + V heads 8-15 pumped into attn(c)
  attn  : scoresT[Sk,Sq] bf16 row-tiled head pair -> exp (ACT, fp8 pairs)
          OT[1+dv, Sq] += [1|V_h].T @ expT  (DoubleRow, row 0 = denom)
          epilogue: recip_fast -> gpsimd bcast -> DVE mul -> SBUF dma
  final : out = LN(concatT.T @ WpT + q_res) * scale + offset
"""

import os
from contextlib import ExitStack

import numpy as np

import concourse.bass as bass
import concourse.tile as tile
from concourse import bacc, mybir
from concourse._compat import with_exitstack
from concourse.bass_utils import run_bass_kernel_spmd

B, S, D = 4, 2048, 1024
H, DK, DV = 16, 64, 64
F = H * DV            # 1024 flattened head dim (== H*DK)
N_CORES = 8
SQ = S // 2           # 1024 queries per core
SK = S                # 2048 keys per core
P = 128
KD = D // P           # 8 contraction chunks over d_model
KDP = KD // 2         # 4 DoubleRow pair-chunks over d_model
P2 = NF2 = 4          # NF//2 DoubleRow pair-chunks over f
NF = F // P           # 8 head-pair chunks
NSK = SK // P         # 16 key chunks
TEMP = float(np.sqrt(D))
EPS = 1e-9

F32 = mybir.dt.float32
BF16 = mybir.dt.bfloat16
F8 = mybir.dt.float8e4
DR = mybir.MatmulPerfMode.DoubleRow

# sk indices at which one pumped proj sliver is emitted (2 DR matmuls
# per sliver: small slivers shrink the PE-FIFO window trapped behind a
# PSUM-buffer wait and relax the proj-psum -> cast pipeline)
PUMP_SKS = tuple(range(16))
# last attn: final chains go after sk4 (the previous call's deferred
# epilogue writes the ot_sb columns they read at sks 1-4)
PUMP_SKS_LAST = (5, 8, 11, 14)

LAST_RESULT = None    # BassKernelResults of the most recent kernel() call

# set per-call from the actual inputs: when scale==1 and offset==0 the two
# trailing elementwise ops are identities and are compiled out
_TRIVIAL_LN = False


@with_exitstack
def _mha_kernel(ctx: ExitStack, tc: tile.TileContext, out_ap, ins):
    nc = tc.nc
    AF = mybir.ActivationFunctionType
    ALU = mybir.AluOpType

    xq_r = ins["qT"].rearrange("(c p) s -> p c s", p=P)
    xk_r = ins["kT"].rearrange("(c p) s -> p c s", p=P)
    xv_r = ins["vT"].rearrange("(c p) s -> p c s", p=P)

    resident = ctx.enter_context(tc.tile_pool(name="resident", bufs=1))
    # V_all with a ones column appended per head: [sk_part, sk, head, 65]
    v_sb = resident.tile([P, NSK, H, 65], F8)
    # concat.T output of attention: partition = f%128, [128, chunk, q]
    ot_sb = resident.tile([P, NF, SQ], F8)
    # ghost ldweights operand (see ghost() below)
    g128 = resident.tile([1, 128], BF16)

    scps = ctx.enter_context(tc.tile_pool(name="scps", bufs=2, space="PSUM"))
    otps = ctx.enter_context(tc.tile_pool(name="otps", bufs=2, space="PSUM"))
    ktp = ctx.enter_context(tc.tile_pool(name="ktp", bufs=2))
    qtp = ctx.enter_context(tc.tile_pool(name="qtp", bufs=2))
    expp = ctx.enter_context(tc.tile_pool(name="expp", bufs=3))
    rcp = ctx.enter_context(tc.tile_pool(name="rcp", bufs=2))
    bcp = ctx.enter_context(tc.tile_pool(name="bcp", bufs=2))
    oop = ctx.enter_context(tc.tile_pool(name="oop", bufs=2))
    if True:
        kt_tiles = {}
        qt_tiles = {}
        hoisted_sc = {}   # (c, sq) -> pre-emitted scores(0) psum tile
        pending_epi = []  # epilogue parts carried from the c-loop's last attn

        def emit_scores_for(c, sq, sk):
            ktc = kt_tiles[c]
            qtc = qt_tiles[c]
            sc = scps.tile([P, 2, 512], F32, tag="sc", name="sc")
            for hh in range(2):
                base = hh * 64
                nc.tensor.matmul(
                    sc[:, hh, :],
                    lhsT=ktc[base:base + 64, sk * P:(sk + 1) * P],
                    rhs=qtc[base:base + 64, sq * 512:(sq + 1) * 512],
                    start=True,
                    stop=True,
                )
            return sc

        def attn(c, sq, work, pump_sks=PUMP_SKS, sched=None, next_attn=None,
                 prev_epi=None, tail_ghosts=0):
            """Attention for head-pair chunk c, query half sq (512 q).

            work: list of emit-closures (projection slivers) pumped at
            pump_sks (or per the explicit sched).  next_attn=(c', sq')
            pre-emits that call's first scores matmul right after this
            call's last exp so the boundary never stalls the exp stream.
            PV matmuls are emitted one pair-step late so their exp input
            is always already complete when the PE FIFO reaches them.
            prev_epi: the previous attn call's deferred epilogue parts,
            emitted into this call's sk 1-4 slots -- by then the previous
            PV accumulation has long completed, so no DVE instruction
            ever waits at the queue head (a stalled DVE queue delays the
            pumped projections' PSUM->SBUF casts, which back-stalls the
            PE FIFO and flips the exp stream into its slow serial mode).
            Returns this call's own epilogue parts.
            """
            ot_ps = []

            def ghost(n, anchor):
                # standalone weight-loads: ~107ns of PE-pipe activity each
                # (128 cols @ 1.2GHz) at near-zero power; the next real
                # matmul's own LDWEIGHTS overwrites, so they are invisible.
                # Reading the just-consumed exp tile anchors them in time:
                # without a data dependency the Tile scheduler floats them
                # all to the very front of the PE stream.
                if anchor is None:
                    return
                for _ in range(n):
                    nc.tensor.ldweights(weights=anchor[0:1, 0, 0, 0:128])

            def pv(dj, exd):
                if not ot_ps:
                    for _ in range(2):
                        ot_ps.append(otps.tile([128, 512], F32, tag="ot",
                                               name="otp"))
                for hh in range(2):
                    nc.tensor.matmul(
                        ot_ps[hh][0:65, :],
                        lhsT=v_sb[:, 2 * dj:2 * dj + 2, 2 * c + hh, :],
                        rhs=exd[:, hh, :, :],  # [128, 2, 512]
                        start=(dj == 0),
                        stop=(dj == NSK // 2 - 1),
                        perf_mode=DR,
                    )

            sc_prev = hoisted_sc.pop((c, sq), None)
            if sc_prev is None:
                sc_prev = emit_scores_for(c, sq, 0)
            ex = ex_prev = None
            for sk in range(NSK):
                par = sk % 2
                if par == 0:
                    # fp8 exp pairs: chunk parity on its own axis so PV can
                    # contract 256 keys per DoubleRow matmul
                    ex_prev = ex
                    ex = expp.tile([P, 2, 2, 512], F8, tag="ex", name="ex")
                nc.scalar.activation(ex[:, :, par, :], sc_prev, AF.Exp,
                                     scale=1.0 / TEMP)
                if sk + 1 < NSK:
                    sc_prev = emit_scores_for(c, sq, sk + 1)
                elif next_attn is not None:
                    hoisted_sc[next_attn] = emit_scores_for(
                        next_attn[0], next_attn[1], 0)
                if prev_epi and 1 <= sk <= 4:
                    prev_epi.pop(0)()
                if sched is not None:
                    for w in sched.get(sk, []):
                        w()
                elif sk in pump_sks:
                    if work:
                        work.pop(0)()
                    else:
                        ghost(3, ex_prev)
                if par == 1 and sk >= 3:
                    pv(sk // 2 - 1, ex_prev)
            pv(NSK // 2 - 1, ex)
            # keep the PE HAM activity monitor fed through the epilogue
            # window (last attn only) so the tail matmuls start at 2.4GHz
            ghost(tail_ghosts, ex)
            # deferred epilogue: normalize rows 1:65 by denom row 0, no PE
            # involvement (custom DVE ops require partition-0 APs).  Parts
            # A (the only ot_ps readers) go in the next call's sk1/sk2;
            # parts B in sk3/sk4, by which point the gpsimd broadcast is
            # done so the muls never stall the DVE queue.
            st = {}

            def part_a(hh):
                def emit():
                    otb = rcp.tile([65, 512], F32, tag="otb")
                    nc.vector.tensor_copy(otb, ot_ps[hh][0:65, :])
                    rc = rcp.tile([1, 512], F32, tag="rc")
                    nc.vector.reciprocal_approx_fast(rc, otb[0:1, :])
                    bc = bcp.tile([65, 512], F32, tag="bc")
                    nc.gpsimd.partition_broadcast(bc, rc)
                    st[hh] = (otb, bc)
                return emit

            def part_b(hh):
                def emit():
                    otb, bc = st.pop(hh)
                    oo = oop.tile([65, 512], F8, tag="oo")
                    nc.vector.tensor_mul(oo, otb, bc)
                    nc.sync.dma_start(
                        ot_sb[hh * 64:(hh + 1) * 64, c,
                              sq * 512:(sq + 1) * 512],
                        oo[1:65, :],
                    )
                return emit

            return [part_a(0), part_a(1), part_b(0), part_b(1)]

        wkq = ctx.enter_context(tc.tile_pool(name="wkq", bufs=1))
        if True:
            wk = wkq.tile([P, KD, F], F8)
            xk = wkq.tile([P, KD, SK], F8)
            wq = wkq.tile([P, KD, F], F8)
            xq = wkq.tile([P, KD, SQ], F8)

            wv = wkq.tile([P, KD, F], F8)
            xv = wkq.tile([P, KD, SK], F8)
            # Input DMAs split across three independent descriptor rings
            # (each ring executes its transfers serially in FIFO order,
            # occupying its issuing engine for the transfer duration):
            #   sync/HWDGE : the K path (first scores + all sk chunks),
            #                done by ~14us, then free for epilogue traffic
            #   scalar ring: the first V chunks (ACT is idle until the
            #                first exp at ~13us; PV needs these by ~16us)
            #   gpsimd/SWDGE: everything else, by first-use deadline
            wv_r = ins["wvT"].rearrange("(c p) f -> p c f", p=P)
            wk_r = ins["wkT"].rearrange("(c p) f -> p c f", p=P)
            wq_r = ins["wqT"].rearrange("(c p) f -> p c f", p=P)
            nc.sync.dma_start(wk[:, :, 0:P], wk_r[:, :, 0:P])
            nc.sync.dma_start(xk[:, :, 0:512], xk_r[:, :, 0:512])
            g = nc.gpsimd
            g.dma_start(wq[:, :, 0:P], wq_r[:, :, 0:P])
            g.dma_start(xq[:, :, 0:512], xq_r[:, :, 0:512])
            g.dma_start(wv[:, :, 0:512], wv_r[:, :, 0:512])
            g.dma_start(xv[:, :, 0:256], xv_r[:, :, 0:256])
            g.dma_start(xv[:, :, 256:512], xv_r[:, :, 256:512])
            g.dma_start(xk[:, :, 512:1024], xk_r[:, :, 512:1024])
            g.dma_start(xv[:, :, 512:768], xv_r[:, :, 512:768])
            g.dma_start(xk[:, :, 1024:1536], xk_r[:, :, 1024:1536])
            g.dma_start(xv[:, :, 768:1024], xv_r[:, :, 768:1024])
            g.dma_start(xk[:, :, 1536:2048], xk_r[:, :, 1536:2048])
            for j in range(4, 8):
                g.dma_start(xv[:, :, j * 256:(j + 1) * 256],
                            xv_r[:, :, j * 256:(j + 1) * 256])
            g.dma_start(xq[:, :, 512:1024], xq_r[:, :, 512:1024])
            g.dma_start(wv[:, :, 512:1024], wv_r[:, :, 512:1024])
            g.dma_start(wk[:, :, P:F], wk_r[:, :, P:F])
            g.dma_start(wq[:, :, P:F], wq_r[:, :, P:F])

            pps = ctx.enter_context(
                tc.tile_pool(name="projps2", bufs=2, space="PSUM"))
            if True:

                # HAM warmup: ~10 dummy matmuls at t~0 (no DMA dependency)
                # push the PE past the 3.4us activity window so the first
                # real projections run at 2.4GHz instead of the cold 1.2.
                # Its memset is the FIRST DVE instruction so the warm
                # matmuls start ~7us in and the PE is hot when the first
                # k/q projection data lands.
                warm_sb = wkq.tile([P, 512], BF16)
                nc.vector.memset(warm_sb, 0.0)
                warm_ps = pps.tile([P, 512], F32, tag="ps", name="warm")
                for _ in range(10):
                    nc.tensor.matmul(
                        warm_ps[0:64, :],
                        lhsT=warm_sb[0:64, 0:64],
                        rhs=warm_sb[0:64, :],
                        start=True,
                        stop=True,
                    )
                nc.vector.memset(v_sb[:, :, :, 0:1], 1.0)
                nc.vector.memset(g128, 0.0)

                def _slivers(mk_mm, copy_out, nuke=2):
                    # split one 4-DR-matmul psum group into slivers
                    st = {}

                    def sliver(i):
                        def emit():
                            if i == 0:
                                st["ps"] = pps.tile([P, 512], F32,
                                                    tag="ps", name="ps")
                            for j in range(nuke * i, nuke * (i + 1)):
                                mk_mm(st["ps"], j)
                            if i == KDP // nuke - 1:
                                copy_out(st["ps"])
                        return emit
                    return [sliver(i) for i in range(KDP // nuke)]

                def v_group(n, sk):
                    def mk_mm(ps, j):
                        nc.tensor.matmul(
                            ps,
                            lhsT=xv[:, 2 * j:2 * j + 2, sk * P:(sk + 1) * P],
                            rhs=wv[:, 2 * j:2 * j + 2, n * 512:(n + 1) * 512],
                            start=(j == 0),
                            stop=(j == KDP - 1),
                            perf_mode=DR,
                        )

                    def copy_out(ps):
                        nc.vector.tensor_copy(
                            v_sb[:, sk, n * 8:(n + 1) * 8, 1:65],
                            ps.rearrange("p (h e) -> p h e", h=8),
                        )
                    return _slivers(mk_mm, copy_out)

                # (V heads 0-7 are deadline-scheduled into attn(0,0))

                def k_group(c, n):
                    def mk_mm(ps, j):
                        if c not in kt_tiles:
                            kt_tiles[c] = ktp.tile([P, SK], BF16, tag="kt",
                                                   name="ktc")
                        nc.tensor.matmul(
                            ps,
                            lhsT=wk[:, 2 * j:2 * j + 2, c * P:(c + 1) * P],
                            rhs=xk[:, 2 * j:2 * j + 2, n * 512:(n + 1) * 512],
                            start=(j == 0),
                            stop=(j == KDP - 1),
                            perf_mode=DR,
                        )

                    def copy_out(ps):
                        nc.vector.tensor_copy(
                            kt_tiles[c][:, n * 512:(n + 1) * 512], ps)
                    return _slivers(mk_mm, copy_out)

                def q_group(c, n):
                    def mk_mm(ps, j):
                        if c not in qt_tiles:
                            qt_tiles[c] = qtp.tile([P, SQ], BF16, tag="qt",
                                                   name="qtc")
                        nc.tensor.matmul(
                            ps,
                            lhsT=wq[:, 2 * j:2 * j + 2, c * P:(c + 1) * P],
                            rhs=xq[:, 2 * j:2 * j + 2, n * 512:(n + 1) * 512],
                            start=(j == 0),
                            stop=(j == KDP - 1),
                            perf_mode=DR,
                        )

                    def copy_out(ps):
                        nc.vector.tensor_copy(
                            qt_tiles[c][:, n * 512:(n + 1) * 512], ps)
                    return _slivers(mk_mm, copy_out)

                if True:
                    # minimal upfront head: only what attn(0,0)'s first
                    # emitted instructions read
                    for s in k_group(0, 0) + q_group(0, 0):
                        s()
                    for s in v_group(0, 0) + v_group(0, 1):
                        s()
                    # remaining head work, scheduled so each group is
                    # emitted strictly before its first reader's emission
                    # (scores(sk+1) emit at step sk; PV(dj) at step 2dj+3)
                    head_sched = {
                        0: v_group(0, 2) + v_group(0, 3),
                        1: k_group(0, 1),
                        2: v_group(0, 4) + v_group(0, 5),
                        3: v_group(0, 6),
                        4: v_group(0, 7) + k_group(0, 2),
                        5: v_group(0, 8),
                        6: v_group(0, 9),
                        7: v_group(0, 10),
                        8: v_group(0, 11) + k_group(0, 3),
                        9: v_group(0, 12),
                        10: v_group(0, 13),
                        11: v_group(0, 14),
                        12: v_group(0, 15),
                        13: q_group(0, 1),
                    }
                    # main loop: attention on c, proj groups for c+1 and
                    # V n=1 slices pumped into the sk streams (V heads
                    # 8-15 spread over c 1-4 to even out pump pressure)
                    epi = None
                    for c in range(NF - 1):
                        work = []
                        if c == 4:
                            # v chunks 12-15 first: attn(4,0)'s own last
                            # PV pairs read them, so they must be emitted
                            # in its earliest pump slots
                            for j in range(4):
                                work += v_group(1, 12 + j)
                        for n in range(SK // 512):
                            work += k_group(c + 1, n)
                        for n in range(SQ // 512):
                            work += q_group(c + 1, n)
                        if 1 <= c <= 3:
                            for j in range(4):
                                work += v_group(1, 4 * (c - 1) + j)
                        if c == 0:
                            epi = attn(0, 0, [], sched=head_sched,
                                       next_attn=(0, 1))
                            epi = attn(0, 1, work, next_attn=(1, 0),
                                       prev_epi=epi)
                        else:
                            epi = attn(c, 0, work, next_attn=(c, 1),
                                       prev_epi=epi)
                            epi = attn(c, 1, work, next_attn=(c + 1, 0),
                                       prev_epi=epi)
                        for w in work:   # leftovers: consumers are in
                            w()          # later blocks, so safe
                        work.clear()
                    pending_epi.extend(epi)

        # ---------------- final: out proj + residual + layernorm -----------
        # (no separate PSUM pool: the pumped chains borrow the idle proj
        # accumulator banks, the tail chains borrow the idle score banks,
        # so no PSUM pool swap -- and no engine drain -- splits the tail)
        wpp = ctx.enter_context(tc.tile_pool(name="wpp", bufs=1))
        lnc = ctx.enter_context(tc.tile_pool(name="lnc", bufs=1))
        qrp = ctx.enter_context(tc.tile_pool(name="qres", bufs=8))
        lnx = ctx.enter_context(tc.tile_pool(name="lnx", bufs=9))
        lnxn = ctx.enter_context(tc.tile_pool(name="lnxn", bufs=3))
        stp = ctx.enter_context(tc.tile_pool(name="stat", bufs=32))
        if True:
            wp = wpp.tile([P, NF, D], F8)
            nc.sync.dma_start(wp, ins["wpT"].rearrange("(c p) f -> p c f", p=P))
            if not _TRIVIAL_LN:
                scale_sb = lnc.tile([P, 2, 512], BF16)
                nc.sync.dma_start(
                    scale_sb,
                    ins["scale_b"].rearrange("p (a b) -> p a b", a=2))
                offset_sb = lnc.tile([P, 2, 512], BF16)
                nc.sync.dma_start(
                    offset_sb,
                    ins["offset_b"].rearrange("p (a b) -> p a b", a=2))

            parts = {}

            def _final_stats(qc, x):
                stats = stp.tile([P, 2, 6], F32, tag="st", name="st")
                for gsub in range(2):
                    nc.vector.bn_stats(stats[:, gsub, :], x[:, gsub, :])
                mv = stp.tile([P, 2], F32, tag="mv", name="mv")
                nc.vector.bn_aggr(mv, stats)
                parts[qc] = (x, mv)

            def _qr_dma(qc):
                qr = qrp.tile([P, 2, 512], BF16, tag="qr")
                nc.sync.dma_start(
                    qr,
                    ins["qres"][qc * P:(qc + 1) * P, :].rearrange(
                        "p (a b) -> p a b", a=2),
                )
                return qr

            def final_mm(qc):
                # pumped (mid-stream) variant: accumulators borrow the
                # idle proj-psum banks
                def emit():
                    qr = _qr_dma(qc)
                    x = lnx.tile([P, 2, 512], BF16, tag="x", name="x")
                    for d in range(2):
                        fp = pps.tile([P, 512], F32, tag="ps", name="fp")
                        for f in range(NF2):
                            nc.tensor.matmul(
                                fp,
                                lhsT=ot_sb[:, 2 * f:2 * f + 2,
                                           qc * P:(qc + 1) * P],
                                rhs=wp[:, 2 * f:2 * f + 2,
                                       d * 512:(d + 1) * 512],
                                start=(f == 0),
                                stop=(f == NF2 - 1),
                                perf_mode=DR,
                            )
                        nc.vector.tensor_add(x[:, d, :], fp, qr[:, d, :])
                    _final_stats(qc, x)
                return emit

            sqscr = wpp.tile([P, 2, 512], BF16)

            def final_mm_tail(qc, qr, act_stats=False):
                # tail variant: a full [128,2,512] score-psum tile holds
                # both halves, so two qc chains are in flight at once.
                # act_stats: mean/sum-of-squares via activation accum_out
                # on the (now idle) scalar engine for half the chunks, so
                # the DVE stats chain is not the single-file tail pacer.
                x = lnx.tile([P, 2, 512], BF16, tag="x", name="x")
                fp2 = scps.tile([P, 2, 512], F32, tag="sc", name="fp2")
                for d in range(2):
                    for f in range(NF2):
                        nc.tensor.matmul(
                            fp2[:, d, :],
                            lhsT=ot_sb[:, 2 * f:2 * f + 2,
                                       qc * P:(qc + 1) * P],
                            rhs=wp[:, 2 * f:2 * f + 2,
                                   d * 512:(d + 1) * 512],
                            start=(f == 0),
                            stop=(f == NF2 - 1),
                            perf_mode=DR,
                        )
                nc.vector.tensor_add(x, fp2, qr)
                if act_stats:
                    sums = stp.tile([P, 2], F32, tag="sm", name="sm")
                    nc.scalar.activation(sqscr, x, AF.Copy,
                                         accum_out=sums[:, 0:1])
                    nc.scalar.activation(sqscr, x, AF.Square,
                                         accum_out=sums[:, 1:2])
                    parts[qc] = (x, ("sums", sums))
                else:
                    _final_stats(qc, x)

            def final_ln_sums(qc):
                # mean/var from ACT-accumulated sums; E[x]^2 ~ 1e-3 << var
                x, (_, sums) = parts[qc]
                mean = stp.tile([P, 1], F32, tag="mean", name="mean")
                nc.vector.tensor_scalar_mul(mean, sums[:, 0:1], 1.0 / D)
                ex2 = stp.tile([P, 1], F32, tag="ex2", name="ex2")
                nc.vector.tensor_scalar_mul(ex2, sums[:, 1:2], 1.0 / D)
                m2 = stp.tile([P, 1], F32, tag="m2", name="m2")
                nc.vector.tensor_mul(m2, mean, mean)
                v = stp.tile([P, 1], F32, tag="v", name="v")
                nc.vector.tensor_sub(v, ex2, m2)
                nc.vector.tensor_scalar_mul(v, v, float(D) / float(D - 1))
                sd = stp.tile([P, 1], F32, tag="sd", name="sd")
                nc.scalar.activation(sd, v, AF.Sqrt)
                y = stp.tile([P, 1], F32, tag="y", name="y")
                nc.vector.reciprocal(y, sd)
                xn = lnxn.tile([P, 2, 512], BF16, tag="xn", name="xn")
                nc.vector.tensor_scalar(xn, x, mean, y,
                                        ALU.subtract, ALU.mult)
                if not _TRIVIAL_LN:
                    nc.vector.tensor_mul(xn, xn, scale_sb)
                    nc.vector.tensor_add(xn, xn, offset_sb)
                nc.sync.dma_start(
                    out_ap[qc * P:(qc + 1) * P, :],
                    xn.rearrange("p a b -> p (a b)"),
                )

            def final_ln_act(qc):
                # all LN chains run in the tail, after the last exp: rstd
                # via one ACT Sqrt (a single table switch) + DVE reciprocal
                # replaces the 10-op DVE Newton chain
                x, mv = parts[qc]
                v = stp.tile([P, 1], F32, tag="v", name="v")
                nc.vector.tensor_scalar_mul(v, mv[:, 1:2],
                                            float(D) / float(D - 1))
                sd = stp.tile([P, 1], F32, tag="sd", name="sd")
                nc.scalar.activation(sd, v, AF.Sqrt)
                y = stp.tile([P, 1], F32, tag="y", name="y")
                nc.vector.reciprocal(y, sd)
                xn = lnxn.tile([P, 2, 512], BF16, tag="xn", name="xn")
                nc.vector.tensor_scalar(xn, x, mv[:, 0:1], y,
                                        ALU.subtract, ALU.mult)
                if not _TRIVIAL_LN:
                    nc.vector.tensor_mul(xn, xn, scale_sb)
                    nc.vector.tensor_add(xn, xn, offset_sb)
                nc.sync.dma_start(
                    out_ap[qc * P:(qc + 1) * P, :],
                    xn.rearrange("p a b -> p (a b)"),
                )

            # last head-pair chunk: final-proj matmuls+stats for sq0
            # pumped into the second half's sk stream (their DVE work
            # overlaps the ACT-paced attention); sq1's groups run after
            # (their ot_sb columns come from this call's epilogue).  The
            # LN chains all run in the tail where ACT/DVE/PE are free.
            c = NF - 1
            epi = attn(c, 0, [], next_attn=(c, 1), prev_epi=pending_epi)

            work = [final_mm(qc) for qc in range(4)]
            epi = attn(c, 1, work, pump_sks=PUMP_SKS_LAST, prev_epi=epi,
                       tail_ghosts=45)
            for w in work:
                w()
            for e in epi:
                e()
            qrs = {qc: _qr_dma(qc) for qc in range(4, SQ // P)}
            for qc in range(4, SQ // P):
                final_mm_tail(qc, qrs[qc], act_stats=(qc % 2 == 1))
                final_ln_act(qc - 4)
            for qc in range(4, SQ // P):
                if qc % 2 == 1:
                    final_ln_sums(qc)
                else:
                    final_ln_act(qc)


def build_program():
    nc = bacc.Bacc("TRN2", debug=False, target_bir_lowering=False)
    shapes = {
        "qT": ([D, SQ], F8), "kT": ([D, SK], F8), "vT": ([D, SK], F8),
        "qres": ([SQ, D], BF16),
        "wqT": ([D, F], F8), "wkT": ([D, F], F8), "wvT": ([D, F], F8),
        "wpT": ([F, D], F8),
        "scale_b": ([P, D], BF16), "offset_b": ([P, D], BF16),
    }
    ins = {k: nc.dram_tensor(k, shp, dt, kind="ExternalInput").ap()
           for k, (shp, dt) in shapes.items()}
    out = nc.dram_tensor("out", [SQ, D], BF16, kind="ExternalOutput").ap()
    with tile.TileContext(nc) as tc:
        _mha_kernel(tc, out, ins)
    nc.compile()
    return nc


_PROGRAM = None
_PROGRAM_TRIVIAL = None


def _get_program(trivial_ln=None):
    global _PROGRAM, _PROGRAM_TRIVIAL, _TRIVIAL_LN
    if trivial_ln is None:
        trivial_ln = _PROGRAM_TRIVIAL if _PROGRAM is not None else False
    if _PROGRAM is None or _PROGRAM_TRIVIAL != trivial_ln:
        _TRIVIAL_LN = trivial_ln
        _PROGRAM = build_program()
        _PROGRAM_TRIVIAL = trivial_ln
    return _PROGRAM


def make_in_maps(q, k, v, Wq, Wk, Wv, Wp, scale, offset):
    import ml_dtypes
    f = np.float32
    bf = ml_dtypes.bfloat16
    f8 = ml_dtypes.float8_e4m3fn
    q = np.asarray(q, f)
    q16 = q.astype(bf)
    q8 = q.astype(f8)
    k8 = np.asarray(k, f).astype(f8)
    v8 = np.asarray(v, f).astype(f8)
    wqT = np.ascontiguousarray(
        np.asarray(Wq, f).transpose(2, 0, 1).reshape(D, F).astype(f8))
    wkT = np.ascontiguousarray(
        np.asarray(Wk, f).transpose(2, 0, 1).reshape(D, F).astype(f8))
    wvT = np.ascontiguousarray(
        np.asarray(Wv, f).transpose(2, 0, 1).reshape(D, F).astype(f8))
    wpT = np.ascontiguousarray(np.asarray(Wp, f).T.astype(f8))
    scale_b = np.ascontiguousarray(
        np.broadcast_to(np.asarray(scale, f), (P, D)).astype(bf))
    offset_b = np.ascontiguousarray(
        np.broadcast_to(np.asarray(offset, f), (P, D)).astype(bf))
    in_maps = []
    for c in range(N_CORES):
        b, half = divmod(c, 2)
        sl = slice(half * SQ, (half + 1) * SQ)
        in_maps.append({
            "qT": np.ascontiguousarray(q8[b, sl].T),
            "qres": np.ascontiguousarray(q16[b, sl]),
            "kT": np.ascontiguousarray(k8[b].T),
            "vT": np.ascontiguousarray(v8[b].T),
            "wqT": wqT, "wkT": wkT, "wvT": wvT, "wpT": wpT,
            "scale_b": scale_b, "offset_b": offset_b,
        })
    return in_maps


def kernel(q, k, v, Wq, Wk, Wv, Wp, scale, offset):
    global LAST_RESULT
    in_maps = make_in_maps(q, k, v, Wq, Wk, Wv, Wp, scale, offset)
    trivial = bool(
        np.all(np.asarray(scale, np.float32) == 1.0)
        and np.all(np.asarray(offset, np.float32) == 0.0))
    nc = _get_program(trivial)
    res = run_bass_kernel_spmd(nc, in_maps, list(range(N_CORES)))
    LAST_RESULT = res
    out = np.empty((B, S, D), np.float32)
    for c in range(N_CORES):
        b, half = divmod(c, 2)
        out[b, half * SQ:(half + 1) * SQ] = \
            res.results[c]["out"].astype(np.float32)
    return out
